# revision 62
# baseline (speedup 1.0000x reference)
"""Trainium2 Bass kernel for nn_Net_18021682774696 (MTGNN-style GNN).

Strategy: data-parallel over batch B=8 -> 1 batch per NeuronCore.
All three adjacency matrices (static, dy, dyT) are host-normalized into
the rhs layout, scaled by 256, cast to fp8-e4m3 and kept FULLY
SBUF-resident (12 MB) -- zero steady-state HBM traffic.  Propagation is
in "p-form" (p1 = A_hat x, p2 = A_hat p1; alpha/(1-alpha) folded into
the host-side 1x1-conv weights), removing all alpha-start matmuls.
Hop 1 runs the 3 branches as concurrent PE column strips (M=32,
tile_position=(0,32j), bf16 stationaries x fp8 moving).  The hop-2
g2-projection is folded into the p1 transposes (the transpose matmul's
rhs is the g2 weight block instead of identity), so all hop-2 partial
products share output channels and spread over FOUR column strips (12
group-steps/chunk instead of 16 + a separate K=97 matmul).  Layer 0's
hop 1 is k-outer/n-inner so it consumes adjacency chunks as the DMA
delivers them; a0/a1 stream on the sync HWDGE queue and a2 on gpsimd
SWDGE, keeping the scalar(ACT) queue trigger-free so TCN activations
are never stuck behind DMA pacing.  Layernorm stats run on the raw
pre-norm copy (prex); x is transposed raw during hop 2 and normalized
afterwards in transposed space (normalize commutes with transpose).
Skip/end projections are pre-collapsed on the host (endW @ skipW_i).
"""
import sys
import os

sys.path.insert(0, '/opt/trn_rl_repo')

import numpy as np
import ml_dtypes

# ----------------------------------------------------------------------------
# Patches: this container's walrus accepts only ONE sem-wait per instruction.
# Split multi-wait instructions (tile attaches one wait per processor).
# ----------------------------------------------------------------------------
import concourse.bass as bass
import concourse.mybir as mybir
import concourse.tile as tile
from concourse.vector_clock import ScopedClock
from concourse.bass_utils import run_bass_kernel_spmd


def _drain_and_barrier_split(self, tick_clock, wait_clock):
    nc = self.nc
    drain_inst = nc.sync.drain()
    wait_clock.add_sem_waits(
        drain_inst.ins, ScopedClock({None: tick_clock.global_clock})
    )
    waits = list(drain_inst.ins.sync_info.on_wait)
    if len(waits) > 1:
        si = drain_inst.ins.sync_info
        si.on_wait = [waits[0]]
        drain_inst.ins.sync_info = si
        for w in waits[1:]:
            d2 = nc.sync.drain()
            d2.ins.sync_info = mybir.SyncInfo(on_wait=[w], on_update=[])

    nc.all_engine_barrier()
    assert self.sems is not None
    popped = nc._tile_sem_poison_stack.pop()
    assert popped is self._sem_poison
    nc.clear_and_free_semaphores(list(self.sems.allocated().values()))
    nc.all_engine_barrier()


tile.TileContext._drain_and_barrier = _drain_and_barrier_split

_orig_postorder = tile.postorder_instruction_blocks
_split_counter = [0]


def _split_multi_waits(ordered, start_bb_name, postordered_blocks):
    for bb_name, insts in ordered.items():
        new_list = []
        for inst in insts:
            si = getattr(inst, 'sync_info', None)
            waits = list(si.on_wait) if si is not None else []
            if len(waits) > 1:
                for w in waits[:-1]:
                    _split_counter[0] += 1
                    nop = mybir.InstNoOp(
                        name=f"I-waitsplit-{_split_counter[0]}", ins=[], outs=[])
                    nop.engine = inst.engine
                    nop.sync_info = mybir.SyncInfo(on_wait=[w], on_update=[])
                    new_list.append(nop)
                si.on_wait = [waits[-1]]
                inst.sync_info = si
            new_list.append(inst)
        ordered[bb_name] = new_list
    return _orig_postorder(ordered, start_bb_name, postordered_blocks)


tile.postorder_instruction_blocks = _split_multi_waits

# ----------------------------------------------------------------------------
# Model constants (hardcoded from the problem spec)
# ----------------------------------------------------------------------------
B, N, C, H, S, T = 8, 2048, 32, 128, 256, 12
LAYERS, NUM_TCN, GDEP = 3, 2, 2
ALPHA, EPS = 0.05, 1e-5
BF16 = mybir.dt.bfloat16
F8 = mybir.dt.float8e4
F32 = mybir.dt.float32
KCH = N // 128          # 16 contraction chunks
NCH = N // 512          # 4 psum n-chunks
NBF = ml_dtypes.bfloat16
NF8 = ml_dtypes.float8_e4m3fn
SCL = 256.0             # adjacency fp8 scale
PSC = 1.0 / SCL         # psum unscale

_prog_cache = {}


def _build(has_affine: bool):
    nc = bass.Bass(trn_type="TRN2", name="gnn_mp")
    ts, AF, ALU = bass.ts, mybir.ActivationFunctionType, mybir.AluOpType

    # ---- DRAM I/O ----
    adj = [nc.dram_tensor(f"adj{j}", [N, N], F8, kind="ExternalInput")
           for j in range(3)]                      # 0 static, 1 dy, 2 dyT
    x0_d = nc.dram_tensor("x0", [C, N], BF16, kind="ExternalInput")
    x0T_d = nc.dram_tensor("x0T", [128, KCH * C], BF16, kind="ExternalInput")
    embs_d = nc.dram_tensor("embs", [96, N], BF16, kind="ExternalInput")
    id16_d = nc.dram_tensor("id16", [96, 32], BF16, kind="ExternalInput")
    tcnW_d = nc.dram_tensor("tcnW", [H, LAYERS * 4 * H], BF16, kind="ExternalInput")
    tcnB_d = nc.dram_tensor("tcnB", [H, LAYERS * 4], F32, kind="ExternalInput")
    g1s_d = nc.dram_tensor("g1s", [96, LAYERS * 32], BF16, kind="ExternalInput")
    g2s_d = nc.dram_tensor("g2s", [97, LAYERS * 32], BF16, kind="ExternalInput")
    ga_d = nc.dram_tensor("ga", [32, LAYERS * 32], BF16, kind="ExternalInput")
    ew_d = nc.dram_tensor("ew", [H, LAYERS * T], BF16, kind="ExternalInput")
    ewe_d = nc.dram_tensor("ewe", [C, T], BF16, kind="ExternalInput")
    cb_d = nc.dram_tensor("cb", [T, 1], F32, kind="ExternalInput")
    sumI_d = nc.dram_tensor("sumI", [128, 32], BF16, kind="ExternalInput")
    gbias_d = nc.dram_tensor("gbias", [32, LAYERS], F32, kind="ExternalInput")
    if has_affine:
        nw_d = nc.dram_tensor("nw", [C, LAYERS * N], BF16, kind="ExternalInput")
        nb_d = nc.dram_tensor("nb", [C, LAYERS * N], BF16, kind="ExternalInput")
    out_d = nc.dram_tensor("out", [T, N], F32, kind="ExternalOutput")

    with tile.TileContext(nc) as tc:
        with (
            tc.tile_pool(name="adjp", bufs=1) as adjp,
            tc.tile_pool(name="cst", bufs=1) as cst,
            tc.tile_pool(name="wk", bufs=1) as wk,
            tc.tile_pool(name="sc", bufs=2) as scp,
            tc.tile_pool(name="hc", bufs=1) as hcp,
            tc.tile_pool(name="ps", bufs=8, space="PSUM") as ps,
        ):
            # ---- persistent SBUF ----
            a0 = adjp.tile([128, KCH, N], F8, name="a0")
            a1 = adjp.tile([128, KCH, N], F8, name="a1")
            a2 = adjp.tile([128, KCH, N], F8, name="a2")
            hid = cst.tile([H, N], BF16, name="hid")        # x rows 0:32, embs 32:128
            id16 = cst.tile([96, 32], BF16, name="id16")    # 16*I blocks
            tcnW = cst.tile([H, LAYERS * 4 * H], BF16, name="tcnW")
            tcnB = cst.tile([H, LAYERS * 4], F32, name="tcnB")
            g1s = cst.tile([96, LAYERS * 32], BF16, name="g1s")
            g2s = cst.tile([97, LAYERS * 32], BF16, name="g2s")
            ga = cst.tile([32, LAYERS * 32], BF16, name="ga")
            ew = cst.tile([H, LAYERS * T], BF16, name="ew")
            ewe = cst.tile([C, T], BF16, name="ewe")
            cb = cst.tile([T, 1], F32, name="cb")
            ones1 = cst.tile([1, 128], F32, name="ones1")
            ones32f = cst.tile([32, 1], F32, name="ones32f")
            sumI = cst.tile([128, 32], BF16, name="sumI")
            gbias = cst.tile([32, LAYERS], F32, name="gbias")
            epsCN2 = cst.tile([1, 1], F32, name="epsCN2")
            if has_affine:
                nw = cst.tile([C, LAYERS * N], BF16, name="nw")
                nb = cst.tile([C, LAYERS * N], BF16, name="nb")

            x_wc = wk.tile([128, KCH, 32], BF16, name="x_wc")
            p1_wc = [wk.tile([128, KCH, 32], BF16, name=f"p1wc{j}")
                     for j in range(3)]
            stack1 = wk.tile([96, N], BF16, name="stack1")
            s4 = wk.tile([128, N], BF16, name="s4")
            prex = wk.tile([32, N], BF16, name="prex")
            out_acc = wk.tile([T, N], F32, name="out_acc")
            sums = wk.tile([32, 2 * NCH], F32, name="sums")
            stat = wk.tile([1, 4], F32, name="stat")
            bc128 = wk.tile([128, 2], F32, name="bc128")
            eps_t = wk.tile([1, 1], F32, name="eps_t")

            # ---- small constants first (gpsimd queue); adjacencies split
            # over the sync/scalar/gpsimd queues in consumption order ----
            nc.sync.dma_start(hid[0:C, :], x0_d[:])
            nc.sync.dma_start(hid[C:H, :], embs_d[:])
            nc.sync.dma_start(x_wc[:].rearrange("p k c -> p (k c)"), x0T_d[:])
            nc.sync.dma_start(tcnW[:], tcnW_d[:])
            nc.sync.dma_start(tcnB[:], tcnB_d[:])
            nc.sync.dma_start(id16[:], id16_d[:])
            nc.sync.dma_start(g1s[:], g1s_d[:])
            nc.sync.dma_start(g2s[:], g2s_d[:])
            nc.sync.dma_start(ga[:], ga_d[:])
            nc.sync.dma_start(ew[:], ew_d[:])
            nc.sync.dma_start(ewe[:], ewe_d[:])
            nc.sync.dma_start(cb[:], cb_d[:])
            nc.sync.dma_start(sumI[:], sumI_d[:])
            nc.sync.dma_start(gbias[:], gbias_d[:])
            if has_affine:
                nc.sync.dma_start(nw[:], nw_d[:])
                nc.sync.dma_start(nb[:], nb_d[:])
            nc.vector.memset(ones1[:], 1.0)
            nc.vector.memset(eps_t[:], EPS)
            nc.vector.memset(ones32f[:], 1.0)
            nc.vector.memset(epsCN2[:], EPS * (C * N) ** 2)
            nc.vector.memset(out_acc[:], 0.0)
            # a0/a1 on the sync HWDGE queue, a2 on gpsimd SWDGE; the
            # scalar(ACT) queue carries NO dma triggers -- trigger pacing
            # there would block the TCN activations behind the whole load
            for k in range(KCH):
                nc.sync.dma_start(a0[:, k, :], adj[0][ts(k, 128), :])
                nc.sync.dma_start(a1[:, k, :], adj[1][ts(k, 128), :])
                nc.gpsimd.dma_start(a2[:, k, :], adj[2][ts(k, 128), :])

            adjs = [a0, a1, a2]

            def prop_step(stationaries, chunk_hook=None, dve_epi=None,
                          post_hook=None):
                """One propagation hop, n-outer / k-inner; 3 branch strips
                run concurrently as PE column strips.  post_hook(n) emits PE
                work that depends on chunk n-1's epilogue -- after chunk n's
                groups, so the cross-engine copy has a chunk of slack."""
                pts = []
                for n in range(NCH):
                    if chunk_hook:
                        chunk_hook(n)
                    pt = ps.tile([128, 512], F32, tag="ps", name=f"pt{n}")
                    for ki in range(KCH):
                        for j in range(3):
                            nc.tensor.matmul(
                                pt[32 * j:32 * j + 32, :],
                                stationaries[j][:, ki, :],
                                adjs[j][:, ki, ts(n, 512)],
                                start=(ki == 0), stop=(ki == KCH - 1),
                                tile_position=(0, 32 * j))
                    if dve_epi:
                        dve_epi(n, pt)
                    if post_hook:
                        post_hook(n)
                    pts.append(pt)
                return pts

            def transpose_grp(src_ap_fn, dst, tpos_row, q):
                """dst[:, 4q:4q+4, :] <- transpose of src cols 512q..+512."""
                tp = ps.tile([128, 128], F32, tag="ps", name="tp")
                for r in range(4):
                    k = 4 * q + r
                    nc.tensor.matmul(
                        tp[:, 32 * r:32 * r + 32],
                        src_ap_fn(k), id16[tpos_row:tpos_row + 32, :],
                        start=True, stop=True, tile_position=(tpos_row, 0))
                nc.vector.tensor_copy(
                    dst[:, 4 * q:4 * q + 4, :].rearrange("p a b -> p (a b)"),
                    tp[:])

            def transpose_grp3(srcs_row_dst, q):
                """Branch-interleaved projected transposes: consecutive
                matmuls target different PE row groups so each LDWEIGHTS
                overlaps the previous matmul instead of serializing behind
                it.  rhs is a per-branch [32,32] projection (identity for a
                plain transpose)."""
                tps = []
                for _ in srcs_row_dst:
                    tps.append(ps.tile([128, 128], F32, tag="ps", name="tp"))
                for r in range(4):
                    for idx, (src_fn, row, dst, rhs) in enumerate(srcs_row_dst):
                        k = 4 * q + r
                        nc.tensor.matmul(
                            tps[idx][:, 32 * r:32 * r + 32],
                            src_fn(k), rhs,
                            start=True, stop=True, tile_position=(row, 0))
                for idx, (src_fn, row, dst, rhs) in enumerate(srcs_row_dst):
                    nc.vector.tensor_copy(
                        dst[:, 4 * q:4 * q + 4, :].rearrange(
                            "p a b -> p (a b)"),
                        tps[idx][:])

            # ================= layers =================
            for i in range(LAYERS):
                wf1 = tcnW[:, ts(i * 4 + 0, H)]
                wg1 = tcnW[:, ts(i * 4 + 1, H)]
                wf2 = tcnW[:, ts(i * 4 + 2, H)]
                wg2 = tcnW[:, ts(i * 4 + 3, H)]
                bf1 = tcnB[:, i * 4 + 0:i * 4 + 1]
                bg1 = tcnB[:, i * 4 + 1:i * 4 + 2]
                bf2 = tcnB[:, i * 4 + 2:i * 4 + 3]
                bg2 = tcnB[:, i * 4 + 3:i * 4 + 4]

                htc = [None] * NCH
                h2cs = [None] * NCH

                def tcn_wave(u, chunks):
                    wf, wg = (wf1, wg1) if u == 0 else (wf2, wg2)
                    bf, bg = (bf1, bg1) if u == 0 else (bf2, bg2)
                    for n in chunks:
                        src_ap = hid[:, ts(n, 512)] if u == 0 else htc[n][:]
                        pf = ps.tile([H, 512], F32, tag="ps", name="pf")
                        pg = ps.tile([H, 512], F32, tag="ps", name="pg")
                        nc.tensor.matmul(pf[:], wf, src_ap, start=True, stop=True)
                        nc.tensor.matmul(pg[:], wg, src_ap, start=True, stop=True)
                        tf = scp.tile([H, 512], BF16, tag="tf", name="tf")
                        tg = scp.tile([H, 512], BF16, tag="tg", name="tg")
                        nc.scalar.activation(tf[:], pf[:], AF.Tanh, bias=bf)
                        nc.scalar.activation(tg[:], pg[:], AF.Sigmoid, bias=bg)
                        ht = hcp.tile([H, 512], BF16, tag=f"htc{n}", name="ht")
                        # layer 0: gpsimd is busy generating a2's DMA
                        # descriptors -- a tensor op there would stall the load
                        eng = nc.vector if i == 0 else nc.gpsimd
                        eng.tensor_mul(ht[:], tf[:], tg[:])
                        if u == 0:
                            htc[n] = ht
                        else:
                            h2cs[n] = ht

                def copy1(n, pt):
                    # last chunk in 4 column pieces on alternating engines so
                    # the first hop-2 transpose only waits for its own piece
                    if n == NCH - 1:
                        for q in range(4):
                            sl = bass.ds(512 * n + 128 * q, 128)
                            if q % 2 == 0:
                                nc.scalar.activation(stack1[:, sl],
                                                     pt[0:96, ts(q, 128)],
                                                     AF.Copy, scale=PSC)
                            else:
                                nc.vector.tensor_scalar_mul(
                                    stack1[:, sl], pt[0:96, ts(q, 128)], PSC)
                    else:
                        nc.vector.tensor_scalar_mul(stack1[:, ts(n, 512)],
                                                    pt[0:96, :], PSC)

                def transposes1(m):
                    transpose_grp3(
                        [(lambda k, j=j: stack1[32 * j:32 * j + 32,
                                                ts(k, 128)],
                          32 * j, p1_wc[j],
                          g2s[32 * j:32 * j + 32, ts(i, 32)])
                         for j in range(3)], m)

                # ---- TCN unit 1 + propagation step 1 ----
                tcn_wave(0, [0, 1])

                if i == 0:
                    # layer 0 is paced by the adjacency DMA: run k-outer /
                    # n-inner so chunk k's work happens as it arrives and
                    # only chunk 15's 4 groups remain after the load tail.
                    # All TCN psums are allocated before the long-lived hop
                    # psums so the pool rotation never blocks mid-loop.
                    tcn_wave(0, [2, 3])
                    pts0 = [ps.tile([128, 512], F32, tag="ps", name=f"pt{n}")
                            for n in range(NCH)]
                    for ki in range(KCH):
                        for n in range(NCH):
                            for j in range(3):
                                nc.tensor.matmul(
                                    pts0[n][32 * j:32 * j + 32, :],
                                    x_wc[:, ki, :],
                                    adjs[j][:, ki, ts(n, 512)],
                                    start=(ki == 0), stop=(ki == KCH - 1),
                                    tile_position=(0, 32 * j))
                    for n in range(NCH):
                        copy1(n, pts0[n])
                    tcn_wave(1, [0, 1])
                    for m in range(NCH):
                        transposes1(m)
                else:
                    def hook1(n):
                        if n in (1, 2):
                            tcn_wave(0, [n + 1])

                    def post1(n):
                        if n >= 1:
                            transposes1(n - 1)

                    prop_step([x_wc, x_wc, x_wc], chunk_hook=hook1,
                              dve_epi=copy1, post_hook=post1)

                    # ---- TCN unit 2 + prop step 2 + mlp/stats ----
                    tcn_wave(1, [0, 1])
                    tcn_wave(1, [2])
                    transposes1(NCH - 1)
                pm = [None] * NCH

                def mlp_block(n):
                    pmn = ps.tile([32, 512], F32, tag="ps", name=f"pm{n}")
                    pm[n] = pmn
                    nc.tensor.matmul(pmn[:], ga[:, ts(i, 32)],
                                     hid[0:32, ts(n, 512)], start=True, stop=False)
                    nc.tensor.matmul(pmn[:], g1s[:, ts(i, 32)],
                                     stack1[:, ts(n, 512)], start=False, stop=False)
                    nc.tensor.matmul(pmn[:], sumI[:],
                                     s4[:, ts(n, 512)], start=False, stop=True)
                    pk = ps.tile([T, 512], F32, tag="ps", name="pk")
                    nc.tensor.matmul(pk[:], ew[:, ts(i, T)], h2cs[n][:],
                                     start=True, stop=True)
                    # raw pre-norm x (+ conv bias) out of psum; stats and the
                    # transpose use the SBUF copy (normalize applied later).
                    # Stats ops come first: they are on the layer-end
                    # critical path, the out_acc accumulation is not.
                    nc.vector.tensor_scalar(
                        out=prex[:, ts(n, 512)], in0=pmn[:],
                        scalar1=gbias[:, i:i + 1], scalar2=1.0,
                        op0=ALU.add, op1=ALU.mult)
                    nc.vector.tensor_reduce(sums[:, n:n + 1],
                                            prex[:, ts(n, 512)],
                                            mybir.AxisListType.X, ALU.add)
                    sq = scp.tile([32, 512], BF16, tag="tmp", name="sq")
                    nc.scalar.activation(sq[:], prex[:, ts(n, 512)], AF.Square,
                                         accum_out=sums[:, NCH + n:NCH + n + 1])
                    nc.vector.tensor_add(out_acc[:, ts(n, 512)],
                                         out_acc[:, ts(n, 512)], pk[:])

                def hook2(n):
                    if n == 1 and i == 0:
                        tcn_wave(1, [2])
                    if n == 1:
                        tcn_wave(1, [3])
                    if n >= 1:
                        mlp_block(n - 1)

                def xregen(n):
                    if i < LAYERS - 1 and not has_affine:
                        transpose_grp(
                            lambda k: prex[0:32, ts(k, 128)], x_wc, 0, n)

                def post2(n):
                    if n >= 1:
                        xregen(n - 1)

                # hop 2: the g2 projection is folded into the transposed
                # stationaries, so all 3 branches share output channels and
                # the 48 (branch, chunk) units spread over 4 column strips.
                for n in range(NCH):
                    hook2(n)
                    pt = ps.tile([128, 512], F32, tag="ps", name=f"q{n}")
                    for t2 in range(12):
                        for s in range(4):
                            u = 4 * t2 + s
                            k, j = divmod(u, 3)
                            nc.tensor.matmul(
                                pt[32 * s:32 * s + 32, :],
                                p1_wc[j][:, k, :],
                                adjs[j][:, k, ts(n, 512)],
                                start=(t2 == 0), stop=(t2 == 11),
                                tile_position=(0, 32 * s))
                    nc.vector.tensor_copy(s4[:, ts(n, 512)], pt[:])
                    post2(n)
                mlp_block(NCH - 1)
                xregen(NCH - 1)

                # dummy sqrt: pull the ACT table swap off the critical path
                nc.scalar.activation(stat[:, 3:4], stat[:, 3:4], AF.Sqrt,
                                     bias=eps_t[:])

                # ---- layernorm scalar chain ----
                ptot = ps.tile([1, 2 * NCH], F32, tag="ps", name="ptot")
                nc.tensor.matmul(ptot[:], ones32f[:], sums[:], start=True, stop=True)
                nc.vector.tensor_reduce(stat[:, 0:1], ptot[:, 0:NCH],
                                        mybir.AxisListType.X, ALU.add)
                nc.vector.tensor_reduce(stat[:, 1:2], ptot[:, NCH:2 * NCH],
                                        mybir.AxisListType.X, ALU.add)
                nc.vector.tensor_mul(stat[:, 2:3], stat[:, 0:1], stat[:, 0:1])
                jp0 = ps.tile([1, 1], F32, tag="ps", name="jp0")
                nc.tensor.matmul(jp0[:], eps_t[:], stat[:, 2:3],
                                 start=True, stop=True)
                nc.vector.scalar_tensor_tensor(
                    out=stat[:, 1:2], in0=stat[:, 1:2], scalar=float(C * N),
                    in1=stat[:, 2:3], op0=ALU.mult, op1=ALU.subtract)
                nc.scalar.activation(stat[:, 1:2], stat[:, 1:2], AF.Sqrt,
                                     bias=epsCN2[:])
                jp = ps.tile([1, 1], F32, tag="ps", name="jp")
                nc.tensor.matmul(jp[:], eps_t[:], stat[:, 1:2],
                                 start=True, stop=True)
                nc.vector.reciprocal(stat[:, 1:2], stat[:, 1:2])
                nc.vector.tensor_scalar_mul(stat[:, 2:3], stat[:, 1:2], float(C * N))
                nc.tensor.matmul(jp[:], eps_t[:], stat[:, 2:3],
                                 start=True, stop=True)
                nc.vector.scalar_tensor_tensor(
                    out=stat[:, 3:4], in0=stat[:, 0:1], scalar=-1.0,
                    in1=stat[:, 1:2], op0=ALU.mult, op1=ALU.mult)
                pb = ps.tile([128, 2], F32, tag="ps", name="pb")
                nc.tensor.matmul(pb[:], ones1[:], stat[:, 2:4], start=True, stop=True)
                nc.scalar.activation(bc128[:], pb[:], AF.Copy)

                # ---- normalize + relu -> hid[0:32] and x_wc ----
                if not has_affine:
                    for n in range(2):
                        nc.scalar.activation(
                            hid[0:32, ts(n, 512)], prex[:, ts(n, 512)],
                            AF.Relu, scale=bc128[0:32, 0:1],
                            bias=bc128[0:32, 1:2])
                    if i < LAYERS - 1:
                        xf0 = x_wc[:, 0, :]
                        nc.scalar.activation(
                            xf0, xf0, AF.Relu, scale=bc128[:, 0:1],
                            bias=bc128[:, 1:2])
                        xfr = x_wc[:, 1:KCH, :].rearrange("p k c -> p (k c)")
                        nc.scalar.activation(
                            xfr, xfr, AF.Relu, scale=bc128[:, 0:1],
                            bias=bc128[:, 1:2])
                    for n in range(2, NCH):
                        nc.scalar.activation(
                            hid[0:32, ts(n, 512)], prex[:, ts(n, 512)],
                            AF.Relu, scale=bc128[0:32, 0:1],
                            bias=bc128[0:32, 1:2])
                else:
                    for n in range(NCH):
                        tmp = scp.tile([32, 512], BF16, tag="tmp", name="tmp")
                        nc.vector.tensor_scalar(
                            out=tmp[:], in0=prex[:, ts(n, 512)],
                            scalar1=bc128[0:32, 0:1], scalar2=bc128[0:32, 1:2],
                            op0=ALU.mult, op1=ALU.add)
                        nc.vector.tensor_mul(tmp[:], tmp[:],
                                             nw[:, bass.ds(i * N + n * 512, 512)])
                        nc.vector.tensor_add(tmp[:], tmp[:],
                                             nb[:, bass.ds(i * N + n * 512, 512)])
                        nc.vector.tensor_scalar_max(hid[0:32, ts(n, 512)],
                                                    tmp[:], 0.0)
                    if i < LAYERS - 1:
                        for n in range(NCH):
                            transpose_grp(lambda k: hid[0:32, ts(k, 128)],
                                          x_wc, 0, n)

                # table-restoring tanh off the hid-relu path; still ahead of
                # the next layer's TCN activations on the ACT queue
                nc.scalar.activation(stat[:, 3:4], stat[:, 3:4], AF.Tanh)

            # ---- final head: out = out_acc + EWE @ x3 + cb ----
            for n in range(NCH):
                pk = ps.tile([T, 512], F32, tag="ps", name="pk")
                nc.tensor.matmul(pk[:], ewe[:], hid[0:32, ts(n, 512)],
                                 start=True, stop=True)
                nc.vector.scalar_tensor_tensor(
                    out=out_acc[:, ts(n, 512)], in0=pk[:], scalar=cb[:],
                    in1=out_acc[:, ts(n, 512)], op0=ALU.add, op1=ALU.add)
            nc.sync.dma_start(out_d[:], out_acc[:])

    return nc


def _prep(inputs):
    """Host-side preprocessing -> per-core input maps."""
    f32 = np.float32
    x = inputs['x'].astype(f32).reshape(B, C, N)
    dy = inputs['dy_graph'].astype(f32)
    S_ = inputs['static_graph'].astype(f32)
    sp = inputs['spatial_emb'].astype(f32).reshape(B, 32, N)
    td = inputs['temporal_d_emb'].astype(f32).reshape(B, 32, N)
    tw = inputs['temporal_w_emb'].astype(f32).reshape(B, 32, N)

    al, s = np.float32(ALPHA), np.float32(1.0 - ALPHA)
    # rhs layouts [w, v]: M0 = (S^T + I)/r0[v], scaled by SCL, fp8
    r0 = S_.sum(1) + 1.0
    adj0 = ((S_.T + np.eye(N, dtype=f32)) * (SCL / r0)[None, :]).astype(NF8)
    adj1 = np.empty((B, N, N), NF8)
    adj2 = np.empty((B, N, N), NF8)
    for b in range(B):
        d = dy[b]
        r1 = d.sum(1) + 1.0
        r2 = d.sum(0) + 1.0
        dT = np.ascontiguousarray(d.T)
        adj1[b] = ((dT + np.eye(N, dtype=f32)) * (SCL / r1)[None, :]).astype(NF8)
        adj2[b] = ((d + np.eye(N, dtype=f32)) * (SCL / r2)[None, :]).astype(NF8)

    id16 = np.zeros((96, 32), f32)
    for j in range(3):
        id16[32 * j:32 * j + 32] = np.eye(32)
    id16 = id16.astype(NBF)

    # TCN weights: lhsT = W^T laid out [cin, (layer,unit,fg)*cout]
    tcnW = np.zeros((H, LAYERS * 4 * H), f32)
    tcnB = np.zeros((H, LAYERS * 4), f32)
    for i in range(LAYERS):
        for u in range(NUM_TCN):
            tcnW[:, (i * 4 + 2 * u) * H:(i * 4 + 2 * u + 1) * H] = \
                inputs['enc_Wf'][i, u].astype(f32).T
            tcnW[:, (i * 4 + 2 * u + 1) * H:(i * 4 + 2 * u + 2) * H] = \
                inputs['enc_Wg'][i, u].astype(f32).T
            tcnB[:, i * 4 + 2 * u] = inputs['enc_bf'][i, u].astype(f32)
            tcnB[:, i * 4 + 2 * u + 1] = inputs['enc_bg'][i, u].astype(f32)

    # p-form mlp weights:
    #   newx = sum_j [(W0+a W1+a W2) x + s(W1+a W2) p1_j + s^2 W2 p2_j] + b
    gW = [inputs['g0_W'].astype(f32), inputs['g1_W'].astype(f32),
          inputs['g2_W'].astype(f32)]
    gB = [inputs['g0_b'].astype(f32), inputs['g1_b'].astype(f32),
          inputs['g2_b'].astype(f32)]
    g1s = np.zeros((96, LAYERS * 32), f32)
    g2s = np.zeros((97, LAYERS * 32), f32)
    ga = np.zeros((32, LAYERS * 32), f32)
    for i in range(LAYERS):
        for k in range(3):   # branch k: 0 static, 1 dy, 2 dyT
            W0 = gW[k][i][:, 0:32]
            W1 = gW[k][i][:, 32:64]
            W2 = gW[k][i][:, 64:96]
            ga[:, 32 * i:32 * i + 32] += (W0 + al * W1 + al * W2).T
            g1s[32 * k:32 * k + 32, 32 * i:32 * i + 32] = (s * (W1 + al * W2)).T
            g2s[32 * k:32 * k + 32, 32 * i:32 * i + 32] = (s * s * W2).T
            g2s[96, 32 * i:32 * i + 32] += gB[k][i]
        ga[:, 32 * i:32 * i + 32] += np.eye(32, dtype=f32)   # residual

    endW = inputs['end_W'].astype(f32)
    ew = np.zeros((H, LAYERS * T), f32)
    for i in range(LAYERS):
        ew[:, i * T:(i + 1) * T] = (endW @ inputs['skip_W'][i].astype(f32)).T
    ewe = (endW @ inputs['skipE_W'].astype(f32)).T
    cb = (endW @ (inputs['skip_b'].astype(f32).sum(0)
                  + inputs['skipE_b'].astype(f32))
          + inputs['end_b'].astype(f32)).reshape(T, 1)

    nw = inputs['norm_w'].astype(f32).reshape(LAYERS, C, N)
    nbb = inputs['norm_b'].astype(f32).reshape(LAYERS, C, N)
    has_affine = not (np.all(nw == 1.0) and np.all(nbb == 0.0))

    sumI = (np.tile(np.eye(32, dtype=f32), (4, 1)) / SCL).astype(NBF)
    gbias = np.stack([gB[0][i] + gB[1][i] + gB[2][i]
                      for i in range(LAYERS)], 1).astype(f32)

    shared = {
        "adj0": adj0, "id16": id16,
        "tcnW": tcnW.astype(NBF), "tcnB": tcnB,
        "g1s": g1s.astype(NBF), "g2s": g2s.astype(NBF),
        "ga": ga.astype(NBF), "sumI": sumI, "gbias": gbias,
        "ew": ew.astype(NBF), "ewe": ewe.astype(NBF), "cb": cb,
    }
    if has_affine:
        shared["nw"] = np.concatenate([nw[i] for i in range(LAYERS)], 1).astype(NBF)
        shared["nb"] = np.concatenate([nbb[i] for i in range(LAYERS)], 1).astype(NBF)

    in_maps = []
    for b in range(B):
        m = dict(shared)
        m["embs"] = np.concatenate([sp[b], td[b], tw[b]], 0).astype(NBF)
        m["x0"] = x[b].astype(NBF)
        m["x0T"] = np.ascontiguousarray(
            x[b].T.reshape(KCH, 128, C).transpose(1, 0, 2).reshape(
                128, KCH * C)).astype(NBF)
        m["adj1"] = adj1[b]
        m["adj2"] = adj2[b]
        in_maps.append(m)
    return in_maps, has_affine


LAST_EXEC_NS = None


def _install_profile_hook():
    import types
    import antenv
    if 'antenv.axon_hooks' not in sys.modules:
        mod = types.ModuleType('antenv.axon_hooks')
        holder = {}
        mod.set_axon_ntff_profile_hook = lambda h: holder.__setitem__('h', h)
        mod.get_axon_ntff_profile_hook = lambda: holder.get('h')
        sys.modules['antenv.axon_hooks'] = mod
        antenv.axon_hooks = mod
        from trn_agent_boot.trn_boot import _ntff_profile_via_ctypes
        mod.set_axon_ntff_profile_hook(
            _ntff_profile_via_ctypes('/opt/axon/libaxon_pjrt.so'))
    import concourse.bass_utils as bu
    bu.upload_artifacts = lambda tmpdir: tmpdir


def kernel(**inputs):
    global LAST_EXEC_NS
    in_maps, has_affine = _prep(inputs)
    if has_affine not in _prog_cache:
        _prog_cache[has_affine] = _build(has_affine)
    nc = _prog_cache[has_affine]
    trace = bool(os.environ.get("KERNEL_TRACE"))
    if trace:
        _install_profile_hook()
    res = run_bass_kernel_spmd(nc, in_maps, core_ids=list(range(B)), trace=trace)
    LAST_EXEC_NS = res.exec_time_ns
    out = np.stack([res.results[b]["out"] for b in range(B)], 0)
    return out.reshape(B, T, N, 1).astype(np.float32)
